# revision 45
# baseline (speedup 1.0000x reference)
"""Trainium2 Bass kernel for ActionExtractionHypersphericalResNet.

Pipeline per sample s:
  mu_raw = h @ W_mu + b_mu            kraw = h @ W_k + b_k
  mu = mu_raw / ||mu_raw||            kappa = softplus(kraw) + 1
  w  = 1 - log(zu0*(1-e) + e)/kappa   with e = exp(-2*kappa)
  z  = mu * w
  out = MLP(z)  (32 -> 512 -> 512 -> 32 -> 7, relu between)

Two mathematical identities vs the reference:
  * The Householder reflection of vn = normalize(v) across
    normalize(vn - mu) maps vn exactly onto mu (both unit vectors), so
    z = mu * w and the `v` input is irrelevant.
  * The radial rejection sampler accepts on trial 0 for every input:
    w_t = 1 - log(z)/k >= 1 (since z <= 1, k >= 1), so
    (d-3)*log(w_t) + k*w_t >= 0 >= log(u).  Hence w depends only on
    zu[0] and kappa.

Sharding: pure data parallel, batch split across 8 cores.  h is
pre-transposed on the host so each core streams contraction-major
tiles straight from HBM (no on-chip transposes of the 256 MB tensor).
The encoder runs in transposed orientation (weights stationary,
h streaming at N=512 — stream-bound) and transposes its small [34, 512]
result back per block on the PE.  All matmuls use float32r (measured
1 cyc/row at N>=512 on silicon vs 4+ for float32; ~1e-4 relative
error).  The MLP runs on transposed activations (features on
partitions) with a 3-stage software pipeline (zT+L1 / L2 / L3+L4)
so PSUM evacuations on DVE/ACT hide under PE work.  Outputs are
written in a per-partition-contiguous scrambled layout and
unscrambled on the host.
"""

import numpy as np

TRACE = False
LAST_RESULTS = None

B = 131072
F = 512
D = 32
H = 512
FS = 32
O = 7
NCORES = 8
BC = B // NCORES          # samples per core
NST = 16                  # supertiles per core
ST = BC // NST            # samples per supertile
NT = ST // 128            # 128-sample slices per supertile
NBLK = ST // 512          # 512-sample blocks per supertile


def _build(benc_zero, b4_zero, reps=1, loop_n=0):
    import concourse.bacc as bacc
    import concourse.mybir as mybir
    import concourse.tile as tile
    from contextlib import ExitStack

    dt = mybir.dt
    F32 = dt.float32
    F32R = dt.float32r
    F16 = dt.float16
    AF = mybir.ActivationFunctionType
    OP = mybir.AluOpType
    AX = mybir.AxisListType

    import bass_rust as _bass_rust
    from concourse.hw_specs import get_activation_tables

    class _Bacc(bacc.Bacc):
        # Keep only one activation table viable (it contains every ACT
        # function this kernel uses) so a single LoadActFuncSet is emitted
        # instead of thrashing between Exp- and Ln-first tables.
        def insert_act_table_loads(self):
            has_activation = any(
                isinstance(i, mybir.InstActivation)
                for b in self.main_func.blocks
                for i in b.instructions
            )
            if not has_activation:
                return
            tables = []
            for name, fns in get_activation_tables(self.m.arch).items():
                if name != "natural_log_exp_and_others":
                    fns = set()
                tables.append((name, fns))
            _bass_rust.insert_act_table_loads(self, tables)

    nc = _Bacc("TRN2", target_bir_lowering=False, debug=False,
               num_devices=NCORES)

    hT = nc.dram_tensor("hT", [F, BC], F32R, kind="ExternalInput").ap()
    zu0s = nc.dram_tensor("zu0s", [NST, 128, NT], F32,
                          kind="ExternalInput").ap()
    Wenc = nc.dram_tensor("Wenc", [F, 34], F32R, kind="ExternalInput").ap()
    BENC = nc.dram_tensor("BENC", [34, 1], F32, kind="ExternalInput").ap()
    W1 = nc.dram_tensor("W1", [D, H], F32R, kind="ExternalInput").ap()
    B1s = nc.dram_tensor("B1s", [128, 4], F32, kind="ExternalInput").ap()
    W2 = nc.dram_tensor("W2", [H, H], F32R, kind="ExternalInput").ap()
    B2s = nc.dram_tensor("B2s", [128, 4], F32, kind="ExternalInput").ap()
    W3 = nc.dram_tensor("W3", [H, FS], F32R, kind="ExternalInput").ap()
    B3s = nc.dram_tensor("B3s", [FS, 1], F32, kind="ExternalInput").ap()
    W4 = nc.dram_tensor("W4", [FS, 8], F32R, kind="ExternalInput").ap()
    B4T = nc.dram_tensor("B4T", [8, 1], F32, kind="ExternalInput").ap()
    IDT = nc.dram_tensor("IDT", [128, 128], F32R, kind="ExternalInput").ap()

    mu_s = nc.dram_tensor("mu_s", [NST, 128, NT * 32], F32,
                          kind="ExternalOutput").ap()
    out_s = nc.dram_tensor("out_s", [NST, 8, ST], F32,
                           kind="ExternalOutput").ap()
    kap_s = nc.dram_tensor("kap_s", [NST, 128, NT], F32,
                           kind="ExternalOutput").ap()

    with tile.TileContext(nc) as tc:
        ctx = ExitStack()
        const = ctx.enter_context(tc.tile_pool(name="const", bufs=1))
        htp = ctx.enter_context(tc.tile_pool(name="htp", bufs=4))
        mkp = ctx.enter_context(tc.tile_pool(name="mkp", bufs=2))
        mtp = ctx.enter_context(tc.tile_pool(name="mtp", bufs=2))
        mup = ctx.enter_context(tc.tile_pool(name="mup", bufs=2))
        zp = ctx.enter_context(tc.tile_pool(name="zp", bufs=2))
        outp = ctx.enter_context(tc.tile_pool(name="outp", bufs=2))
        small = ctx.enter_context(tc.tile_pool(name="small", bufs=2))
        x1p = ctx.enter_context(tc.tile_pool(name="x1p", bufs=12))
        x2p = ctx.enter_context(tc.tile_pool(name="x2p", bufs=8))
        ztp = ctx.enter_context(tc.tile_pool(name="ztp", bufs=3))
        x3p = ctx.enter_context(tc.tile_pool(name="x3p", bufs=2))
        psE_p = ctx.enter_context(tc.tile_pool(name="psE_p", bufs=2, space="PSUM"))
        psB_p = ctx.enter_context(tc.tile_pool(name="psB_p", bufs=1, space="PSUM"))
        psZ_p = ctx.enter_context(tc.tile_pool(name="psZ_p", bufs=1, space="PSUM"))
        ps1_p = ctx.enter_context(tc.tile_pool(name="ps1_p", bufs=2, space="PSUM"))
        psL_p = ctx.enter_context(tc.tile_pool(name="psL_p", bufs=2, space="PSUM"))

        # ---- constants / weights ----
        # all on the gpsimd (SWDGE) queue so the sync queue carries only the
        # hT stream and the first encoder matmul starts as early as possible
        wenc_sb = const.tile([128, 4, 34], F32R, name="wenc_sb")
        nc.gpsimd.dma_start(wenc_sb[:], Wenc.rearrange("(a p) c -> p a c", p=128))
        idt = const.tile([128, 128], F32R, name="idt")
        nc.gpsimd.dma_start(idt[:], IDT[:])
        benc_sb = const.tile([34, 1], F32, name="benc_sb")
        nc.gpsimd.dma_start(benc_sb[:], BENC[:])
        w1_sb = const.tile([D, H], F32R, name="w1_sb")
        nc.gpsimd.dma_start(w1_sb[:], W1[:])
        b1_sb = const.tile([128, 4], F32, name="b1_sb")
        nc.gpsimd.dma_start(b1_sb[:], B1s[:])
        w2_sb = const.tile([128, 4, H], F32R, name="w2_sb")
        nc.gpsimd.dma_start(w2_sb[:], W2.rearrange("(a p) c -> p a c", p=128))
        b2_sb = const.tile([128, 4], F32, name="b2_sb")
        nc.gpsimd.dma_start(b2_sb[:], B2s[:])
        w3_sb = const.tile([128, 4, FS], F32R, name="w3_sb")
        nc.gpsimd.dma_start(w3_sb[:], W3.rearrange("(a p) c -> p a c", p=128))
        b3_sb = const.tile([FS, 1], F32, name="b3_sb")
        nc.gpsimd.dma_start(b3_sb[:], B3s[:])
        w4_sb = const.tile([FS, 8], F32R, name="w4_sb")
        nc.gpsimd.dma_start(w4_sb[:], W4[:])
        b4t_sb = const.tile([8, 1], F32, name="b4t_sb")
        nc.gpsimd.dma_start(b4t_sb[:], B4T[:])
        mb2 = const.tile([128, 1], F32, name="mb2")
        nc.vector.memset(mb2[:], -2.0)

        mk_tiles = {}
        z_tiles = {}

        def emit_E(st):
            mk = mkp.tile([128, NT, 33], F32, name="mk")
            mk_tiles[st] = mk
            hT_v = hT.rearrange("(a p) c -> p a c", p=128)

            def back_transpose(mT_prev, blk_prev):
                # f32r, NOT 16-bit: 16-bit transposes lower to standalone
                # LDWEIGHTS + transpose-mode pairs that cost ~1.2us each on
                # silicon vs ~144ns for f32r's fused load.
                psB = psB_p.tile([128, 4, 34], F32R, name="psB")
                for sl in range(4):
                    nc.tensor.transpose(psB[:, sl, :],
                                        mT_prev[:, sl * 128:(sl + 1) * 128],
                                        idt[0:34, 0:34])
                nc.vector.tensor_copy(
                    mk[:, blk_prev * 4:(blk_prev + 1) * 4, :],
                    psB[:, :, 0:33].bitcast(F32))

            prev = None
            for blk in range(NBLK):
                ht = htp.tile([128, 4, 512], F32R, name="ht")
                nc.sync.dma_start(
                    ht[:], hT_v[:, :, st * ST + blk * 512:
                                st * ST + (blk + 1) * 512])
                # encoder in transposed orientation: [mu_raw|kraw]^T [34, 512]
                psT = psE_p.tile([34, 512], F32, name="psE")
                for ki in range(4):
                    nc.tensor.matmul(psT[:], wenc_sb[:, ki, :], ht[:, ki, :],
                                     start=(ki == 0), stop=(ki == 3))
                # transposes of the PREVIOUS block sit between this block's
                # matmuls and evac so the PE never waits on the ACT evac.
                if prev is not None:
                    back_transpose(*prev)
                mT = mtp.tile([34, 512], F32R, name="mT")
                if benc_zero:
                    nc.scalar.copy(mT[:], psT[:])
                else:
                    nc.scalar.activation(mT[:], psT[:], AF.Identity,
                                         bias=benc_sb[:], scale=1.0)
                prev = (mT, blk)
            back_transpose(*prev)

        def emit_S(st):
            mk = mk_tiles[st]
            mk_mu = mk[:, :, 0:32]     # [128, NT, 32] stride-33 view
            mk_k = mk[:, :, 32]        # [128, NT]

            zu0t = small.tile([128, NT], F32, name="zu0t")
            nc.sync.dma_start(zu0t[:], zu0s[st, :, :])

            # softplus(x) = ln(exp(x) + 1) — Softplus has no ACT table here;
            # Exp/Ln/Square live in one table so no table switches.
            ex = small.tile([128, NT], F32, name="ex")
            nc.scalar.activation(ex[:], mk_k, AF.Exp, bias=0.0, scale=1.0)
            sp = small.tile([128, NT], F32, name="sp")
            nc.scalar.activation(sp[:], ex[:], AF.Ln, bias=1.0, scale=1.0)
            kapt = small.tile([128, NT], F32, name="kapt")
            nc.vector.tensor_scalar_add(kapt[:], sp[:], 1.0)
            nc.sync.dma_start(kap_s[st, :, :], kapt[:])

            rk = small.tile([128, NT], F32, name="rk")
            nc.vector.reciprocal(rk[:], kapt[:])
            e = small.tile([128, NT], F32, name="e")
            nc.scalar.activation(e[:], sp[:], AF.Exp, bias=mb2[:, 0:1], scale=-2.0)
            ome = small.tile([128, NT], F32, name="ome")
            nc.vector.tensor_scalar(ome[:], e[:], -1.0, 1.0,
                                    op0=OP.mult, op1=OP.add)
            z0 = small.tile([128, NT], F32, name="z0")
            nc.vector.tensor_tensor(z0[:], zu0t[:], ome[:], op=OP.mult)
            nc.vector.tensor_tensor(z0[:], z0[:], e[:], op=OP.add)
            lz = small.tile([128, NT], F32, name="lz")
            nc.scalar.activation(lz[:], z0[:], AF.Ln, bias=0.0, scale=1.0)
            t1 = small.tile([128, NT], F32, name="t1")
            nc.vector.tensor_tensor(t1[:], lz[:], rk[:], op=OP.mult)
            w = small.tile([128, NT], F32, name="w")
            nc.vector.tensor_scalar(w[:], t1[:], -1.0, 1.0,
                                    op0=OP.mult, op1=OP.add)

            sq = small.tile([128, NT, 32], F32, name="sq")
            nc.scalar.activation(sq[:], mk_mu, AF.Square, bias=0.0, scale=1.0)
            nmu2 = small.tile([128, NT], F32, name="nmu2")
            nc.vector.tensor_reduce(nmu2[:], sq[:], axis=AX.X, op=OP.add)
            # 1/sqrt(x) = exp(-0.5*ln(x)) — avoids the Sqrt ACT table.
            lnm = small.tile([128, NT], F32, name="lnm")
            nc.scalar.activation(lnm[:], nmu2[:], AF.Ln, bias=0.0, scale=1.0)
            rn = small.tile([128, NT], F32, name="rn")
            nc.scalar.activation(rn[:], lnm[:], AF.Exp, bias=0.0, scale=-0.5)
            rnw = small.tile([128, NT], F32, name="rnw")
            nc.vector.tensor_tensor(rnw[:], rn[:], w[:], op=OP.mult)

            mu_t = mup.tile([128, NT, 32], F32, name="mu_t")
            nc.vector.tensor_tensor(
                mu_t[:], mk_mu, rn[:].unsqueeze(2).to_broadcast([128, NT, 32]),
                op=OP.mult)
            z_t = zp.tile([128, NT, 32], F32R, name="z_t")
            z_tiles[st] = z_t
            nc.vector.tensor_tensor(
                z_t[:], mk_mu, rnw[:].unsqueeze(2).to_broadcast([128, NT, 32]),
                op=OP.mult)
            nc.sync.dma_start(mu_s[st, :, :], mu_t[:].rearrange("p t d -> p (t d)"))

        def emit_M(st):
            z_t = z_tiles[st]
            outt = outp.tile([8, ST], F32, name="outt")
            x1s, x2s = {}, {}

            def stage_A(b):      # z transpose + L1
                zps = psZ_p.tile([FS, 512], F32R, name="psZ")
                for sl in range(4):
                    nc.tensor.transpose(zps[:, sl * 128:(sl + 1) * 128],
                                        z_t[:, b * 4 + sl, :], idt[:])
                zsb = ztp.tile([FS, 512], F32R, name="zsb")
                nc.vector.tensor_copy(zsb[:], zps[:])
                x1 = []
                for m in range(4):
                    ps1 = ps1_p.tile([128, 512], F32, name="ps1")
                    nc.tensor.matmul(ps1[:], w1_sb[:, m * 128:(m + 1) * 128],
                                     zsb[:], start=True, stop=True)
                    t = x1p.tile([128, 512], F32R, name="x1t")
                    if m % 2 == 0:
                        nc.vector.tensor_scalar(t[:], ps1[:], b1_sb[:, m:m + 1],
                                                0.0, op0=OP.add, op1=OP.max)
                    else:
                        nc.scalar.activation(t[:], ps1[:], AF.Relu,
                                             bias=b1_sb[:, m:m + 1], scale=1.0)
                    x1.append(t)
                x1s[b] = x1

            def stage_B(b):      # L2
                x1 = x1s.pop(b)
                x2 = []
                for m in range(4):
                    ps2 = psL_p.tile([128, 512], F32, name="psL")
                    for ki in range(4):
                        nc.tensor.matmul(
                            ps2[:], w2_sb[:, ki, m * 128:(m + 1) * 128],
                            x1[ki][:], start=(ki == 0), stop=(ki == 3))
                    t = x2p.tile([128, 512], F32R, name="x2t")
                    if m % 2 == 0:
                        nc.vector.tensor_scalar(t[:], ps2[:], b2_sb[:, m:m + 1],
                                                0.0, op0=OP.add, op1=OP.max)
                    else:
                        nc.scalar.activation(t[:], ps2[:], AF.Relu,
                                             bias=b2_sb[:, m:m + 1], scale=1.0)
                    x2.append(t)
                x2s[b] = x2

            def stage_C(b):      # L3 + L4 + out
                x2 = x2s.pop(b)
                ps3 = psL_p.tile([FS, 512], F32, name="psL")
                for ki in range(4):
                    nc.tensor.matmul(ps3[:], w3_sb[:, ki, :], x2[ki][:],
                                     start=(ki == 0), stop=(ki == 3))
                x3 = x3p.tile([FS, 512], F32R, name="x3sb")
                nc.scalar.activation(x3[:], ps3[:], AF.Relu,
                                     bias=b3_sb[:], scale=1.0)
                ps4 = psL_p.tile([8, 512], F32, name="psL")
                nc.tensor.matmul(ps4[:], w4_sb[:], x3[:], start=True, stop=True)
                dst = outt[:, b * 512:(b + 1) * 512]
                if b4_zero:
                    nc.vector.tensor_copy(dst, ps4[:])
                else:
                    nc.vector.tensor_tensor(
                        dst, ps4[:],
                        b4t_sb[:, 0:1].to_broadcast([8, 512]), op=OP.add)

            for b in range(NBLK):
                stage_A(b)
                if b >= 1:
                    stage_B(b - 1)
                if b >= 2:
                    stage_C(b - 2)
            stage_B(NBLK - 1)
            stage_C(NBLK - 2)
            stage_C(NBLK - 1)
            nc.sync.dma_start(out_s[st, :, :], outt[:])

        def emit_all():
            for rep in range(reps):
                mk_tiles.clear()
                z_tiles.clear()
                emit_E(0)
                for st in range(NST):
                    emit_S(st)
                    if st + 1 < NST:
                        emit_E(st + 1)
                    emit_M(st)

        if loop_n:
            with tc.For_i(0, loop_n, 1):
                emit_all()
        else:
            emit_all()

        ctx.close()
    nc.compile()
    return nc


def kernel(h, v, zu, uu, W_mu, b_mu, W_k, b_k, W1, b1, W2, b2, W3, b3, W4, b4,
           **_unused):
    from concourse.bass_utils import run_bass_kernel_spmd

    h = np.asarray(h, np.float32)
    zu = np.asarray(zu, np.float32)
    Wenc = np.ascontiguousarray(
        np.concatenate([np.asarray(W_mu, np.float32),
                        np.asarray(W_k, np.float32),
                        np.zeros((F, 1), np.float32)], axis=1))
    benc = np.concatenate([np.asarray(b_mu, np.float32),
                           np.asarray(b_k, np.float32)])
    BENC = np.ascontiguousarray(
        np.concatenate([benc, np.zeros(1, np.float32)]).reshape(34, 1))
    B4T = np.ascontiguousarray(
        np.concatenate([np.asarray(b4, np.float32),
                        np.zeros(1, np.float32)]).reshape(8, 1))
    IDT = np.eye(128, dtype=np.float32)

    benc_zero = not benc.any()
    b4_zero = not np.asarray(b4).any()

    nc = _build(benc_zero, b4_zero)

    shared = {
        "Wenc": Wenc, "BENC": BENC,
        "W1": np.ascontiguousarray(np.asarray(W1, np.float32)),
        "B1s": np.ascontiguousarray(np.asarray(b1, np.float32).reshape(4, 128).T),
        "W2": np.ascontiguousarray(np.asarray(W2, np.float32)),
        "B2s": np.ascontiguousarray(np.asarray(b2, np.float32).reshape(4, 128).T),
        "W3": np.ascontiguousarray(np.asarray(W3, np.float32)),
        "B3s": np.ascontiguousarray(np.asarray(b3, np.float32).reshape(FS, 1)),
        "W4": np.ascontiguousarray(np.concatenate(
            [np.asarray(W4, np.float32), np.zeros((FS, 1), np.float32)], axis=1)),
        "B4T": B4T, "IDT": IDT,
    }
    in_maps = []
    for c in range(NCORES):
        sl = slice(c * BC, (c + 1) * BC)
        # zu0 pre-shuffled to [st][p][t2] so the device load is contiguous.
        zu0c = zu[0, sl].reshape(NST, NT, 128).transpose(0, 2, 1)
        in_maps.append(dict(
            shared,
            hT=np.ascontiguousarray(h[sl, :].T),
            zu0s=np.ascontiguousarray(zu0c),
        ))

    global LAST_RESULTS
    res = run_bass_kernel_spmd(nc, in_maps, core_ids=list(range(NCORES)),
                               trace=TRACE)
    LAST_RESULTS = res

    outs, mus, kaps = [], [], []
    for r in res.results:
        # unscramble [st][p][t2]... -> sample-major (s = st*ST + t2*128 + p)
        o = r["out_s"].reshape(NST, 8, ST).transpose(0, 2, 1)[:, :, :O]
        outs.append(o.reshape(BC, O))
        m = r["mu_s"].reshape(NST, 128, NT, 32).transpose(0, 2, 1, 3)
        mus.append(m.reshape(BC, 32))
        k = r["kap_s"].reshape(NST, 128, NT).transpose(0, 2, 1)
        kaps.append(k.reshape(BC, 1))
    out = np.ascontiguousarray(np.concatenate(outs, axis=0))
    mu = np.ascontiguousarray(np.concatenate(mus, axis=0))
    kappa = np.ascontiguousarray(np.concatenate(kaps, axis=0))
    return (out, mu, kappa)


# revision 46
# speedup vs baseline: 1.0146x; 1.0146x over previous
"""Trainium2 Bass kernel for ActionExtractionHypersphericalResNet.

Pipeline per sample s:
  mu_raw = h @ W_mu + b_mu            kraw = h @ W_k + b_k
  mu = mu_raw / ||mu_raw||            kappa = softplus(kraw) + 1
  w  = 1 - log(zu0*(1-e) + e)/kappa   with e = exp(-2*kappa)
  z  = mu * w
  out = MLP(z)  (32 -> 512 -> 512 -> 32 -> 7, relu between)

Two mathematical identities vs the reference:
  * The Householder reflection of vn = normalize(v) across
    normalize(vn - mu) maps vn exactly onto mu (both unit vectors), so
    z = mu * w and the `v` input is irrelevant.
  * The radial rejection sampler accepts on trial 0 for every input:
    w_t = 1 - log(z)/k >= 1 (since z <= 1, k >= 1), so
    (d-3)*log(w_t) + k*w_t >= 0 >= log(u).  Hence w depends only on
    zu[0] and kappa.

Sharding: pure data parallel, batch split across 8 cores.  h is
pre-transposed on the host so each core streams contraction-major
tiles straight from HBM (no on-chip transposes of the 256 MB tensor).
The encoder runs in transposed orientation (weights stationary,
h streaming at N=512 — stream-bound) and transposes its small [34, 512]
result back per block on the PE.  All matmuls use float32r (measured
1 cyc/row at N>=512 on silicon vs 4+ for float32; ~1e-4 relative
error).  The MLP runs on transposed activations (features on
partitions) with a 3-stage software pipeline (zT+L1 / L2 / L3+L4)
so PSUM evacuations on DVE/ACT hide under PE work.  Outputs are
written in a per-partition-contiguous scrambled layout and
unscrambled on the host.
"""

import numpy as np

TRACE = False
LAST_RESULTS = None

B = 131072
F = 512
D = 32
H = 512
FS = 32
O = 7
NCORES = 8
BC = B // NCORES          # samples per core
NST = 16                  # supertiles per core
ST = BC // NST            # samples per supertile
NT = ST // 128            # 128-sample slices per supertile
NBLK = ST // 512          # 512-sample blocks per supertile


def _build(benc_zero, b4_zero, reps=1, loop_n=0):
    import concourse.bacc as bacc
    import concourse.mybir as mybir
    import concourse.tile as tile
    from contextlib import ExitStack

    dt = mybir.dt
    F32 = dt.float32
    F32R = dt.float32r
    F16 = dt.float16
    AF = mybir.ActivationFunctionType
    OP = mybir.AluOpType
    AX = mybir.AxisListType

    import bass_rust as _bass_rust
    from concourse.hw_specs import get_activation_tables

    class _Bacc(bacc.Bacc):
        # Keep only one activation table viable (it contains every ACT
        # function this kernel uses) so a single LoadActFuncSet is emitted
        # instead of thrashing between Exp- and Ln-first tables.
        def insert_act_table_loads(self):
            has_activation = any(
                isinstance(i, mybir.InstActivation)
                for b in self.main_func.blocks
                for i in b.instructions
            )
            if not has_activation:
                return
            tables = []
            for name, fns in get_activation_tables(self.m.arch).items():
                if name != "natural_log_exp_and_others":
                    fns = set()
                tables.append((name, fns))
            _bass_rust.insert_act_table_loads(self, tables)

    nc = _Bacc("TRN2", target_bir_lowering=False, debug=False,
               num_devices=NCORES)

    hT = nc.dram_tensor("hT", [F, BC], F32R, kind="ExternalInput").ap()
    zu0s = nc.dram_tensor("zu0s", [NST, 128, NT], F32,
                          kind="ExternalInput").ap()
    Wenc = nc.dram_tensor("Wenc", [F, 34], F32R, kind="ExternalInput").ap()
    BENC = nc.dram_tensor("BENC", [34, 1], F32, kind="ExternalInput").ap()
    W1 = nc.dram_tensor("W1", [D, H], F32R, kind="ExternalInput").ap()
    B1s = nc.dram_tensor("B1s", [128, 4], F32, kind="ExternalInput").ap()
    W2 = nc.dram_tensor("W2", [H, H], F32R, kind="ExternalInput").ap()
    B2s = nc.dram_tensor("B2s", [128, 4], F32, kind="ExternalInput").ap()
    W3 = nc.dram_tensor("W3", [H, FS], F32R, kind="ExternalInput").ap()
    B3s = nc.dram_tensor("B3s", [FS, 1], F32, kind="ExternalInput").ap()
    W4 = nc.dram_tensor("W4", [FS, 8], F32R, kind="ExternalInput").ap()
    B4T = nc.dram_tensor("B4T", [8, 1], F32, kind="ExternalInput").ap()
    IDT = nc.dram_tensor("IDT", [128, 128], F32R, kind="ExternalInput").ap()

    mu_s = nc.dram_tensor("mu_s", [NST, 128, NT * 32], F32,
                          kind="ExternalOutput").ap()
    out_s = nc.dram_tensor("out_s", [NST, 8, ST], F32,
                           kind="ExternalOutput").ap()
    kap_s = nc.dram_tensor("kap_s", [NST, 128, NT], F32,
                           kind="ExternalOutput").ap()

    with tile.TileContext(nc) as tc:
        ctx = ExitStack()
        const = ctx.enter_context(tc.tile_pool(name="const", bufs=1))
        htp = ctx.enter_context(tc.tile_pool(name="htp", bufs=4))
        mkp = ctx.enter_context(tc.tile_pool(name="mkp", bufs=2))
        mtp = ctx.enter_context(tc.tile_pool(name="mtp", bufs=2))
        mup = ctx.enter_context(tc.tile_pool(name="mup", bufs=2))
        zp = ctx.enter_context(tc.tile_pool(name="zp", bufs=2))
        outp = ctx.enter_context(tc.tile_pool(name="outp", bufs=2))
        small = ctx.enter_context(tc.tile_pool(name="small", bufs=2))
        x1p = ctx.enter_context(tc.tile_pool(name="x1p", bufs=12))
        x2p = ctx.enter_context(tc.tile_pool(name="x2p", bufs=16))
        ztp = ctx.enter_context(tc.tile_pool(name="ztp", bufs=3))
        x3p = ctx.enter_context(tc.tile_pool(name="x3p", bufs=2))
        psE_p = ctx.enter_context(tc.tile_pool(name="psE_p", bufs=2, space="PSUM"))
        psB_p = ctx.enter_context(tc.tile_pool(name="psB_p", bufs=1, space="PSUM"))
        psZ_p = ctx.enter_context(tc.tile_pool(name="psZ_p", bufs=1, space="PSUM"))
        ps1_p = ctx.enter_context(tc.tile_pool(name="ps1_p", bufs=2, space="PSUM"))
        psL_p = ctx.enter_context(tc.tile_pool(name="psL_p", bufs=2, space="PSUM"))

        # ---- constants / weights ----
        # all on the gpsimd (SWDGE) queue so the sync queue carries only the
        # hT stream and the first encoder matmul starts as early as possible
        wenc_sb = const.tile([128, 4, 34], F32R, name="wenc_sb")
        nc.gpsimd.dma_start(wenc_sb[:], Wenc.rearrange("(a p) c -> p a c", p=128))
        idt = const.tile([128, 128], F32R, name="idt")
        nc.gpsimd.dma_start(idt[:], IDT[:])
        benc_sb = const.tile([34, 1], F32, name="benc_sb")
        nc.gpsimd.dma_start(benc_sb[:], BENC[:])
        w1_sb = const.tile([D, H], F32R, name="w1_sb")
        nc.gpsimd.dma_start(w1_sb[:], W1[:])
        b1_sb = const.tile([128, 4], F32, name="b1_sb")
        nc.gpsimd.dma_start(b1_sb[:], B1s[:])
        w2_sb = const.tile([128, 4, H], F32R, name="w2_sb")
        nc.gpsimd.dma_start(w2_sb[:], W2.rearrange("(a p) c -> p a c", p=128))
        b2_sb = const.tile([128, 4], F32, name="b2_sb")
        nc.gpsimd.dma_start(b2_sb[:], B2s[:])
        w3_sb = const.tile([128, 4, FS], F32R, name="w3_sb")
        nc.gpsimd.dma_start(w3_sb[:], W3.rearrange("(a p) c -> p a c", p=128))
        b3_sb = const.tile([FS, 1], F32, name="b3_sb")
        nc.gpsimd.dma_start(b3_sb[:], B3s[:])
        w4_sb = const.tile([FS, 8], F32R, name="w4_sb")
        nc.gpsimd.dma_start(w4_sb[:], W4[:])
        b4t_sb = const.tile([8, 1], F32, name="b4t_sb")
        nc.gpsimd.dma_start(b4t_sb[:], B4T[:])
        mb2 = const.tile([128, 1], F32, name="mb2")
        nc.vector.memset(mb2[:], -2.0)

        mk_tiles = {}
        z_tiles = {}
        outt_tiles = {}
        x1s, x2s, x3s = {}, {}, {}

        def emit_E(st):
            mk = mkp.tile([128, NT, 33], F32, name="mk")
            mk_tiles[st] = mk
            hT_v = hT.rearrange("(a p) c -> p a c", p=128)

            def back_transpose(mT_prev, blk_prev):
                # f32r, NOT 16-bit: 16-bit transposes lower to standalone
                # LDWEIGHTS + transpose-mode pairs that cost ~1.2us each on
                # silicon vs ~144ns for f32r's fused load.
                psB = psB_p.tile([128, 4, 34], F32R, name="psB")
                for sl in range(4):
                    nc.tensor.transpose(psB[:, sl, :],
                                        mT_prev[:, sl * 128:(sl + 1) * 128],
                                        idt[0:34, 0:34])
                nc.vector.tensor_copy(
                    mk[:, blk_prev * 4:(blk_prev + 1) * 4, :],
                    psB[:, :, 0:33].bitcast(F32))

            prev = None
            for blk in range(NBLK):
                ht = htp.tile([128, 4, 512], F32R, name="ht")
                nc.sync.dma_start(
                    ht[:], hT_v[:, :, st * ST + blk * 512:
                                st * ST + (blk + 1) * 512])
                # encoder in transposed orientation: [mu_raw|kraw]^T [34, 512]
                psT = psE_p.tile([34, 512], F32, name="psE")
                for ki in range(4):
                    nc.tensor.matmul(psT[:], wenc_sb[:, ki, :], ht[:, ki, :],
                                     start=(ki == 0), stop=(ki == 3))
                # transposes of the PREVIOUS block sit between this block's
                # matmuls and evac so the PE never waits on the ACT evac.
                if prev is not None:
                    back_transpose(*prev)
                mT = mtp.tile([34, 512], F32R, name="mT")
                if benc_zero:
                    nc.scalar.copy(mT[:], psT[:])
                else:
                    nc.scalar.activation(mT[:], psT[:], AF.Identity,
                                         bias=benc_sb[:], scale=1.0)
                prev = (mT, blk)
            back_transpose(*prev)

        def emit_S(st):
            mk = mk_tiles[st]
            mk_mu = mk[:, :, 0:32]     # [128, NT, 32] stride-33 view
            mk_k = mk[:, :, 32]        # [128, NT]

            zu0t = small.tile([128, NT], F32, name="zu0t")
            nc.sync.dma_start(zu0t[:], zu0s[st, :, :])

            # softplus(x) = ln(exp(x) + 1) — Softplus has no ACT table here;
            # Exp/Ln/Square live in one table so no table switches.
            ex = small.tile([128, NT], F32, name="ex")
            nc.scalar.activation(ex[:], mk_k, AF.Exp, bias=0.0, scale=1.0)
            sp = small.tile([128, NT], F32, name="sp")
            nc.scalar.activation(sp[:], ex[:], AF.Ln, bias=1.0, scale=1.0)
            kapt = small.tile([128, NT], F32, name="kapt")
            nc.vector.tensor_scalar_add(kapt[:], sp[:], 1.0)
            nc.sync.dma_start(kap_s[st, :, :], kapt[:])

            rk = small.tile([128, NT], F32, name="rk")
            nc.vector.reciprocal(rk[:], kapt[:])
            e = small.tile([128, NT], F32, name="e")
            nc.scalar.activation(e[:], sp[:], AF.Exp, bias=mb2[:, 0:1], scale=-2.0)
            ome = small.tile([128, NT], F32, name="ome")
            nc.vector.tensor_scalar(ome[:], e[:], -1.0, 1.0,
                                    op0=OP.mult, op1=OP.add)
            z0 = small.tile([128, NT], F32, name="z0")
            nc.vector.tensor_tensor(z0[:], zu0t[:], ome[:], op=OP.mult)
            nc.vector.tensor_tensor(z0[:], z0[:], e[:], op=OP.add)
            lz = small.tile([128, NT], F32, name="lz")
            nc.scalar.activation(lz[:], z0[:], AF.Ln, bias=0.0, scale=1.0)
            t1 = small.tile([128, NT], F32, name="t1")
            nc.vector.tensor_tensor(t1[:], lz[:], rk[:], op=OP.mult)
            w = small.tile([128, NT], F32, name="w")
            nc.vector.tensor_scalar(w[:], t1[:], -1.0, 1.0,
                                    op0=OP.mult, op1=OP.add)

            sq = small.tile([128, NT, 32], F32, name="sq")
            nc.scalar.activation(sq[:], mk_mu, AF.Square, bias=0.0, scale=1.0)
            nmu2 = small.tile([128, NT], F32, name="nmu2")
            nc.vector.tensor_reduce(nmu2[:], sq[:], axis=AX.X, op=OP.add)
            # 1/sqrt(x) = exp(-0.5*ln(x)) — avoids the Sqrt ACT table.
            lnm = small.tile([128, NT], F32, name="lnm")
            nc.scalar.activation(lnm[:], nmu2[:], AF.Ln, bias=0.0, scale=1.0)
            rn = small.tile([128, NT], F32, name="rn")
            nc.scalar.activation(rn[:], lnm[:], AF.Exp, bias=0.0, scale=-0.5)
            rnw = small.tile([128, NT], F32, name="rnw")
            nc.vector.tensor_tensor(rnw[:], rn[:], w[:], op=OP.mult)

            mu_t = mup.tile([128, NT, 32], F32, name="mu_t")
            nc.vector.tensor_tensor(
                mu_t[:], mk_mu, rn[:].unsqueeze(2).to_broadcast([128, NT, 32]),
                op=OP.mult)
            z_t = zp.tile([128, NT, 32], F32R, name="z_t")
            z_tiles[st] = z_t
            nc.vector.tensor_tensor(
                z_t[:], mk_mu, rnw[:].unsqueeze(2).to_broadcast([128, NT, 32]),
                op=OP.mult)
            nc.sync.dma_start(mu_s[st, :, :], mu_t[:].rearrange("p t d -> p (t d)"))

        def emit_M(st, pend):
            z_t = z_tiles[st]
            outt = outp.tile([8, ST], F32, name="outt")
            outt_tiles[st] = outt
            zsbs = {}

            def stage_zT(b):
                zps = psZ_p.tile([FS, 512], F32R, name="psZ")
                for sl in range(4):
                    nc.tensor.transpose(zps[:, sl * 128:(sl + 1) * 128],
                                        z_t[:, b * 4 + sl, :], idt[:])
                zsb = ztp.tile([FS, 512], F32R, name="zsb")
                nc.vector.tensor_copy(zsb[:], zps[:])
                zsbs[b] = zsb

            def stage_L1(b):
                zsb = zsbs.pop(b)
                x1 = []
                for m in range(4):
                    ps1 = ps1_p.tile([128, 512], F32, name="ps1")
                    nc.tensor.matmul(ps1[:], w1_sb[:, m * 128:(m + 1) * 128],
                                     zsb[:], start=True, stop=True)
                    t = x1p.tile([128, 512], F32R, name="x1t")
                    if m % 2 == 0:
                        nc.vector.tensor_scalar(t[:], ps1[:], b1_sb[:, m:m + 1],
                                                0.0, op0=OP.add, op1=OP.max)
                    else:
                        nc.scalar.activation(t[:], ps1[:], AF.Relu,
                                             bias=b1_sb[:, m:m + 1], scale=1.0)
                    x1.append(t)
                x1s[b] = x1

            def stage_B(b):
                x1 = x1s.pop(b)
                x2 = []
                for m in range(4):
                    ps2 = psL_p.tile([128, 512], F32, name="psL")
                    for ki in range(4):
                        nc.tensor.matmul(
                            ps2[:], w2_sb[:, ki, m * 128:(m + 1) * 128],
                            x1[ki][:], start=(ki == 0), stop=(ki == 3))
                    t = x2p.tile([128, 512], F32R, name="x2t")
                    if m % 2 == 0:
                        nc.vector.tensor_scalar(t[:], ps2[:], b2_sb[:, m:m + 1],
                                                0.0, op0=OP.add, op1=OP.max)
                    else:
                        nc.scalar.activation(t[:], ps2[:], AF.Relu,
                                             bias=b2_sb[:, m:m + 1], scale=1.0)
                    x2.append(t)
                x2s[(st, b)] = x2

            for b in range(NBLK):
                stage_zT(b)
                # previous supertile's L3 rides here: pure MMs covering the
                # zsb copy the next L1 would otherwise stall on
                if pend is not None:
                    stage_L3(pend, b)
                stage_L1(b)
                if pend is not None:
                    stage_L4(pend, b)
            if pend is not None:
                nc.sync.dma_start(out_s[pend, :, :], outt_tiles.pop(pend)[:])
            for b in range(NBLK):
                stage_B(b)

        def stage_L3(stc, b):
            x2 = x2s.pop((stc, b))
            ps3 = psL_p.tile([FS, 512], F32, name="psL")
            for ki in range(4):
                nc.tensor.matmul(ps3[:], w3_sb[:, ki, :], x2[ki][:],
                                 start=(ki == 0), stop=(ki == 3))
            x3 = x3p.tile([FS, 512], F32R, name="x3sb")
            nc.scalar.activation(x3[:], ps3[:], AF.Relu,
                                 bias=b3_sb[:], scale=1.0)
            x3s[(stc, b)] = x3

        def stage_L4(stc, b):
            x3 = x3s.pop((stc, b))
            ps4 = psL_p.tile([8, 512], F32, name="psL")
            nc.tensor.matmul(ps4[:], w4_sb[:], x3[:], start=True, stop=True)
            dst = outt_tiles[stc][:, b * 512:(b + 1) * 512]
            if b4_zero:
                nc.vector.tensor_copy(dst, ps4[:])
            else:
                nc.vector.tensor_tensor(
                    dst, ps4[:],
                    b4t_sb[:, 0:1].to_broadcast([8, 512]), op=OP.add)

        def flush_C(stc):
            for b in range(NBLK):
                stage_L3(stc, b)
                stage_L4(stc, b)
            nc.sync.dma_start(out_s[stc, :, :], outt_tiles.pop(stc)[:])

        def emit_all():
            for rep in range(reps):
                mk_tiles.clear()
                z_tiles.clear()
                outt_tiles.clear()
                x1s.clear(); x2s.clear(); x3s.clear()
                emit_E(0)
                for st in range(NST):
                    emit_S(st)
                    if st + 1 < NST:
                        emit_E(st + 1)
                    emit_M(st, st - 1 if st > 0 else None)
                flush_C(NST - 1)

        if loop_n:
            with tc.For_i(0, loop_n, 1):
                emit_all()
        else:
            emit_all()

        ctx.close()
    nc.compile()
    return nc


def kernel(h, v, zu, uu, W_mu, b_mu, W_k, b_k, W1, b1, W2, b2, W3, b3, W4, b4,
           **_unused):
    from concourse.bass_utils import run_bass_kernel_spmd

    h = np.asarray(h, np.float32)
    zu = np.asarray(zu, np.float32)
    Wenc = np.ascontiguousarray(
        np.concatenate([np.asarray(W_mu, np.float32),
                        np.asarray(W_k, np.float32),
                        np.zeros((F, 1), np.float32)], axis=1))
    benc = np.concatenate([np.asarray(b_mu, np.float32),
                           np.asarray(b_k, np.float32)])
    BENC = np.ascontiguousarray(
        np.concatenate([benc, np.zeros(1, np.float32)]).reshape(34, 1))
    B4T = np.ascontiguousarray(
        np.concatenate([np.asarray(b4, np.float32),
                        np.zeros(1, np.float32)]).reshape(8, 1))
    IDT = np.eye(128, dtype=np.float32)

    benc_zero = not benc.any()
    b4_zero = not np.asarray(b4).any()

    nc = _build(benc_zero, b4_zero)

    shared = {
        "Wenc": Wenc, "BENC": BENC,
        "W1": np.ascontiguousarray(np.asarray(W1, np.float32)),
        "B1s": np.ascontiguousarray(np.asarray(b1, np.float32).reshape(4, 128).T),
        "W2": np.ascontiguousarray(np.asarray(W2, np.float32)),
        "B2s": np.ascontiguousarray(np.asarray(b2, np.float32).reshape(4, 128).T),
        "W3": np.ascontiguousarray(np.asarray(W3, np.float32)),
        "B3s": np.ascontiguousarray(np.asarray(b3, np.float32).reshape(FS, 1)),
        "W4": np.ascontiguousarray(np.concatenate(
            [np.asarray(W4, np.float32), np.zeros((FS, 1), np.float32)], axis=1)),
        "B4T": B4T, "IDT": IDT,
    }
    in_maps = []
    for c in range(NCORES):
        sl = slice(c * BC, (c + 1) * BC)
        # zu0 pre-shuffled to [st][p][t2] so the device load is contiguous.
        zu0c = zu[0, sl].reshape(NST, NT, 128).transpose(0, 2, 1)
        in_maps.append(dict(
            shared,
            hT=np.ascontiguousarray(h[sl, :].T),
            zu0s=np.ascontiguousarray(zu0c),
        ))

    global LAST_RESULTS
    res = run_bass_kernel_spmd(nc, in_maps, core_ids=list(range(NCORES)),
                               trace=TRACE)
    LAST_RESULTS = res

    outs, mus, kaps = [], [], []
    for r in res.results:
        # unscramble [st][p][t2]... -> sample-major (s = st*ST + t2*128 + p)
        o = r["out_s"].reshape(NST, 8, ST).transpose(0, 2, 1)[:, :, :O]
        outs.append(o.reshape(BC, O))
        m = r["mu_s"].reshape(NST, 128, NT, 32).transpose(0, 2, 1, 3)
        mus.append(m.reshape(BC, 32))
        k = r["kap_s"].reshape(NST, 128, NT).transpose(0, 2, 1)
        kaps.append(k.reshape(BC, 1))
    out = np.ascontiguousarray(np.concatenate(outs, axis=0))
    mu = np.ascontiguousarray(np.concatenate(mus, axis=0))
    kappa = np.ascontiguousarray(np.concatenate(kaps, axis=0))
    return (out, mu, kappa)
